# revision 26
# baseline (speedup 1.0000x reference)
"""Trainium2 Bass kernel for the DenoisingModule (non-local attention block).

Math (see reference):
    theta = Wt @ x + bt            [B, 128, HW]
    phi   = Wp @ x + bp            [B, 128, HW]
    f     = theta^T @ phi / 16     [B, HW, HW]
    fh    = softmax(f, axis=0)     (over the BATCH axis - PyTorch legacy dim=0)
    den   = fh @ x^T               [B, C, HW]
    out   = den + (Wc @ den + bc)  = (I + Wc) @ den + bc

Sharding: the softmax couples all 8 batch elements at each (n, m) position,
so batch-parallel would need a 64MB cross-device all-reduce.  Instead we
shard the *n* axis (rows of f / output pixels): each of the 8 cores owns
n in [k*512, (k+1)*512), holds full x, and the softmax is fully local.
No collectives at all; host slices inputs and concatenates outputs.

v3 (this file), on top of the bf16 datapath of v2:
  - softmax reduction tree moved to the otherwise-idle Pool engine
    (pairs 0-2 and their combines; only the late pair 6+7 and the final
    add stay on DVE), full-width [128,2048] ops;
  - the fh=fexp*R muls and the ln/exp reciprocal run full-width;
  - den matmuls for chunk k are emitted with a 2-slot lag into chunk
    k+1's stream so PE never stalls on the softmax tail;
  - startup DMAs are pair-batched (xs+xn interleaved per batch pair)
    and phi pair 0 is hoisted before theta pairs 1-3 so the first f
    matmul fires ~8us earlier;
  - conv bias-adds moved from DVE to Act (idle at the tail).
Engine budget per m-chunk: Act ~25us (16 exps + ln/exp + phi copies),
DVE ~22us (muls + den spills + late tree), PE ~24us (f + den + phi),
Pool ~21us (early tree).

The installed walrus rejects any engine/DMA instruction carrying more
than one semaphore wait ("Too many sync wait commands"), but Tile's
sem-assignment emits up to 4.  _split_excess_waits() legalizes the
scheduled program post-hoc by hoisting excess waits onto single-wait
EventSemaphore instructions inserted just before, on the same engine
queue (applied on the hardware path only; CoreSim runs the pre-split
program).
"""

import sys

import numpy as np

B = 8
C = 256
D = C // 2  # 128
HW = 4096
NCORES = 8
NLOC = HW // NCORES  # 512 n-columns per core
MC = 512  # m-chunk size
NCHUNK = HW // MC  # 8
P = 128

TRACE = False
TRACE_CORES = None
TRACE_DIR = None
LAST = {}

# pool-size knobs
XN_BUFS = 4    # paired [P,2,2,MC] tiles
XT_BUFS = 10
PHI_BUFS = 2
FEXP_BUFS = 2
PSA_BUFS = 2
PSD_BUFS = 2
OUT_BUFS = 2
DEN_LAG = 2    # den(c,b) emitted at slot b+DEN_LAG of chunk c+1
# scheduling knobs
TAIL_SLOTS = (0,)      # slots where LN/R pieces are emitted (1 or 2 pieces)
PHI_DVE = (0, 2)       # phi pairs whose copy runs on DVE (rest on Act)
MUL_SLOT = None        # slot where the muls are emitted (None: with last LN/R)

_prog = None


def _ensure_path():
    try:
        import concourse  # noqa: F401
    except ImportError:
        for p in ("/opt/trn_rl_repo", "/root/.axon_site/_ro/trn_rl_repo"):
            if p not in sys.path:
                sys.path.insert(0, p)
        import concourse  # noqa: F401


def _build(reps=1):
    from contextlib import ExitStack

    import concourse.bass as bass
    import concourse.tile as tile
    from concourse import mybir

    f32 = mybir.dt.float32
    f32r = mybir.dt.float32r
    bf16 = mybir.dt.bfloat16
    AF = mybir.ActivationFunctionType

    nc = bass.Bass(trn_type="TRN2", target_bir_lowering=False, debug=False)

    xs_h = nc.dram_tensor("xs", [B, 2, P, NLOC], bf16, kind="ExternalInput")
    xn_h = nc.dram_tensor("xn", [B, 2, P, HW], bf16, kind="ExternalInput")
    # xt laid out [B, chunk, s, p, c] so one DMA fetches a whole chunk
    xt_h = nc.dram_tensor("xt", [B, NCHUNK, 4, P, C], bf16, kind="ExternalInput")
    wthT_h = nc.dram_tensor("wthT", [C, D], bf16, kind="ExternalInput")
    wphT_h = nc.dram_tensor("wphT", [C, D], bf16, kind="ExternalInput")
    wcT_h = nc.dram_tensor("wcT", [C, C], f32r, kind="ExternalInput")
    bth_h = nc.dram_tensor("bth", [D, 1], f32, kind="ExternalInput")
    bph_h = nc.dram_tensor("bph", [D, 1], f32, kind="ExternalInput")
    bc_h = nc.dram_tensor("bc", [C, 1], f32, kind="ExternalInput")
    out_h = nc.dram_tensor("out", [B, 2, P, NLOC], f32, kind="ExternalOutput")

    FW = 4 * NLOC  # 2048: full free width of an fexp tile

    with tile.TileContext(nc) as tc:
        with ExitStack() as ctx:
            consts = ctx.enter_context(tc.tile_pool(name="consts", bufs=1))
            theta_p = ctx.enter_context(tc.tile_pool(name="theta", bufs=1))
            xs_p = ctx.enter_context(tc.tile_pool(name="xsp", bufs=2))
            xn_p = ctx.enter_context(tc.tile_pool(name="xnp", bufs=XN_BUFS))
            xt_p = ctx.enter_context(tc.tile_pool(name="xtp", bufs=XT_BUFS))
            phi_p = ctx.enter_context(tc.tile_pool(name="phip", bufs=PHI_BUFS))
            fexp_p = ctx.enter_context(tc.tile_pool(name="fexpp", bufs=FEXP_BUFS))
            smx_p = ctx.enter_context(tc.tile_pool(name="smxp", bufs=1))
            den_p = ctx.enter_context(tc.tile_pool(name="denp", bufs=1))
            out_p = ctx.enter_context(tc.tile_pool(name="outp", bufs=OUT_BUFS))
            psA = ctx.enter_context(tc.tile_pool(name="psA", bufs=PSA_BUFS, space="PSUM"))
            psD = ctx.enter_context(tc.tile_pool(name="psD", bufs=PSD_BUFS, space="PSUM"))

            # ---- constants (emitted in dependency-criticality order; the
            # first xs/xn pair DMAs are interleaved by emit_rep before the
            # conv constants) ----
            wth_sb = []
            wph_sb = []
            wc_sb = []

            def emit_theta_consts():
                for ck in range(2):
                    t = consts.tile([P, D], bf16, name=f"wth{ck}", tag=f"wth{ck}")
                    nc.sync.dma_start(out=t, in_=wthT_h.ap()[ck * P:(ck + 1) * P, :])
                    wth_sb.append(t)
                t = consts.tile([D, 1], f32, name="bth", tag="bth")
                nc.sync.dma_start(out=t, in_=bth_h.ap()[:, :])
                return t

            def emit_phi_consts():
                for ck in range(2):
                    t = consts.tile([P, D], bf16, name=f"wph{ck}", tag=f"wph{ck}")
                    nc.sync.dma_start(out=t, in_=wphT_h.ap()[ck * P:(ck + 1) * P, :])
                    wph_sb.append(t)
                t = consts.tile([D, 1], f32, name="bph", tag="bph")
                nc.sync.dma_start(out=t, in_=bph_h.ap()[:, :])
                return t

            def emit_conv_consts():
                bc_sb = []
                for ck in range(2):
                    t = consts.tile([P, C], f32r, name=f"wc{ck}", tag=f"wc{ck}")
                    nc.sync.dma_start(out=t, in_=wcT_h.ap()[ck * P:(ck + 1) * P, :])
                    wc_sb.append(t)
                for dk in range(2):
                    t = consts.tile([P, 1], f32, name=f"bc{dk}", tag=f"bc{dk}")
                    nc.sync.dma_start(out=t, in_=bc_h.ap()[dk * P:(dk + 1) * P, :])
                    bc_sb.append(t)
                return bc_sb

            cst = {}

            def emit_rep(rp):
                theta_sb = [None] * B
                xn0_tiles = [None] * 4  # per pair

                def emit_theta_pair(p, first=False):
                    xst = xs_p.tile([P, 2, 2, NLOC], bf16, name=f"{rp}xs{p}", tag="xs")
                    nc.sync.dma_start(
                        out=xst,
                        in_=xs_h.ap()[2 * p:2 * p + 2].transpose([2, 0, 1, 3]))
                    xnt = xn_p.tile([P, 2, 2, MC], bf16, name=f"{rp}xn0_{p}", tag="xn")
                    nc.sync.dma_start(
                        out=xnt,
                        in_=xn_h.ap()[2 * p:2 * p + 2, :, :, 0:MC]
                        .transpose([2, 0, 1, 3]))
                    xn0_tiles[p] = xnt
                    if first and not wth_sb:
                        cst["bth"] = emit_theta_consts()
                        cst["bph"] = emit_phi_consts()
                    ps = psA.tile([P, 2 * NLOC], f32, name=f"{rp}psth{p}", tag="psA")
                    for ck in range(2):
                        for bi in range(2):
                            nc.tensor.matmul(
                                ps[:, bi * NLOC:(bi + 1) * NLOC],
                                wth_sb[ck], xst[:, bi, ck, :],
                                start=(ck == 0), stop=(ck == 1))
                    for bi in range(2):
                        b = 2 * p + bi
                        th = theta_p.tile([D, NLOC], bf16, name=f"{rp}theta{b}",
                                          tag=f"theta{b}")
                        nc.scalar.activation(th, ps[:, bi * NLOC:(bi + 1) * NLOC],
                                             AF.Identity, bias=cst["bth"])
                        theta_sb[b] = th

                den_sb = [None] * B

                def emit_conv(b):
                    # out = (I + Wc) @ den + bc  (f32r matmul, bias-add on Act)
                    ot = out_p.tile([P, 2, NLOC], f32, name=f"{rp}out{b}", tag="out")
                    for dk in range(2):
                        ps = psA.tile([P, 2 * NLOC], f32, name=f"{rp}pso{b}_{dk}",
                                      tag="psA")
                        for ct in range(2):
                            nc.tensor.matmul(
                                ps[:, :NLOC],
                                wc_sb[ct][:, dk * P:(dk + 1) * P],
                                den_sb[b][:, ct * NLOC:(ct + 1) * NLOC],
                                start=(ct == 0), stop=(ct == 1))
                        nc.scalar.activation(ot[:, dk, :], ps[:, :NLOC],
                                             AF.Identity, bias=cst["bc"][dk])
                    nc.sync.dma_start(out=out_h.ap()[b].transpose([1, 0, 2]), in_=ot)

                def emit_den_b(mc, fexp, xt_t, b):
                    psd = psD.tile([P, 2 * NLOC], f32, name=f"{rp}psd{mc}_{b}",
                                   tag="psD")
                    for ct in range(2):
                        for s in range(4):
                            nc.tensor.matmul(
                                psd[:, ct * NLOC:(ct + 1) * NLOC],
                                xt_t[b][:, s, ct * P:(ct + 1) * P],
                                fexp[b][:, s * NLOC:(s + 1) * NLOC],
                                start=(s == 0), stop=(s == 3))
                    if mc == 0:
                        dn = den_p.tile([P, 2 * NLOC], f32r, name=f"{rp}den{b}",
                                        tag=f"den{b}")
                        nc.vector.tensor_copy(dn, psd)
                        den_sb[b] = dn
                    else:
                        nc.vector.tensor_add(den_sb[b], den_sb[b], psd)
                    if mc == NCHUNK - 1:
                        emit_conv(b)

                # startup: pair 0's big DMAs go first, the small constant
                # DMAs are interleaved right behind them, conv constants
                # after theta pair 0 (needed only at the tail).
                emit_theta_pair(0, first=True)
                if "bc" not in cst:
                    cst["bc"] = emit_conv_consts()

                from collections import deque
                den_q = deque()   # (mc, fexp, xt_t, b) awaiting den emission
                prev_tail = None  # emits LN/R piece of the previous chunk
                prev_muls = None  # emits fexp*R muls of the previous chunk

                for mc in range(NCHUNK):
                    m0 = mc * MC
                    phi_sb = []

                    def emit_phi_pair(bp, mc=mc, m0=m0):
                        ps = psA.tile([P, 2 * NLOC], f32, name=f"{rp}psph{mc}_{bp}",
                                      tag="psA")
                        if mc == 0:
                            xnt = xn0_tiles[bp]
                        else:
                            xnt = xn_p.tile([P, 2, 2, MC], bf16,
                                            name=f"{rp}xn{mc}_{bp}", tag="xn")
                            nc.sync.dma_start(
                                out=xnt,
                                in_=xn_h.ap()[2 * bp:2 * bp + 2, :, :, m0:m0 + MC]
                                .transpose([2, 0, 1, 3]))
                        for ck in range(2):
                            for bi in range(2):
                                nc.tensor.matmul(
                                    ps[:, bi * MC:(bi + 1) * MC], wph_sb[ck],
                                    xnt[:, bi, ck, :],
                                    start=(ck == 0), stop=(ck == 1))
                        php = phi_p.tile([D, 2, MC], bf16, name=f"{rp}phi{mc}_{bp}",
                                         tag=f"phi{bp}")
                        # copies split across DVE / Act per the PHI_DVE knob
                        if bp in PHI_DVE:
                            nc.vector.tensor_scalar(php, ps, cst["bph"], None,
                                                    mybir.AluOpType.add)
                        else:
                            nc.scalar.activation(php, ps, AF.Identity,
                                                 bias=cst["bph"])
                        phi_sb.append(php)

                    if mc > 0:
                        # all four phi pairs up front: their copies precede
                        # the muls on DVE, so f matmuls are never phi-blocked
                        for bp in range(4):
                            emit_phi_pair(bp)
                        # two oldest queued dens (chunk mc-2's trailing pair):
                        # PE chews these while the phi copies drain.  Only
                        # when the queue holds >8 (i.e. the head is from two
                        # chunks back, whose muls are long emitted)
                        while len(den_q) > 8:
                            emit_den_b(*den_q.popleft())

                    # f~ = theta'^T phi, exp -> fexp [m=128, (s,n) free], bf16.
                    # Softmax tree rides the exp stream: Pool sums pairs 0-2
                    # and combines them; DVE only does the late pair 6+7 and
                    # the final add.  The previous chunk's LN/R + muls are
                    # emitted after slot 0's exps so the next exp stream isn't
                    # queued behind them; den matmuls of the previous chunk
                    # lag DEN_LAG slots so their fexp*R inputs are ready.
                    fexp = []
                    tree = {}
                    xt_t = []
                    for b in range(B):
                        if mc == 0 and b % 2 == 0:
                            emit_phi_pair(b // 2)
                            if b < 6:
                                emit_theta_pair(b // 2 + 1)
                        fe = fexp_p.tile([P, FW], bf16, name=f"{rp}fexp{mc}_{b}",
                                         tag=f"fexp{b}")
                        for sp in range(2):
                            ps = psA.tile([P, 2 * NLOC], f32,
                                          name=f"{rp}psf{mc}_{b}_{sp}", tag="psA")
                            for si in range(2):
                                s = sp * 2 + si
                                nc.tensor.matmul(
                                    ps[:, si * NLOC:(si + 1) * NLOC],
                                    phi_sb[b // 2][:, b % 2, s * P:(s + 1) * P],
                                    theta_sb[b],
                                    start=True, stop=True)
                            nc.scalar.activation(
                                fe[:, sp * 2 * NLOC:(sp + 1) * 2 * NLOC], ps,
                                AF.Exp)
                        fexp.append(fe)
                        # xT tile for this (chunk, batch), spread across slots
                        t = xt_p.tile([P, 4, C], bf16, name=f"{rp}xt{mc}_{b}",
                                      tag="xt")
                        nc.sync.dma_start(
                            out=t, in_=xt_h.ap()[b, mc].transpose([1, 0, 2]))
                        xt_t.append(t)
                        for piece, slot in enumerate(TAIL_SLOTS):
                            if b == slot and prev_tail is not None:
                                prev_tail(piece)
                        if MUL_SLOT is not None and b == MUL_SLOT \
                                and prev_muls is not None:
                            prev_muls()
                        if b == 1:
                            t = smx_p.tile([P, FW], bf16, name=f"{rp}p01_{mc}",
                                           tag="p01")
                            nc.gpsimd.tensor_add(t, fexp[0], fexp[1])
                            tree["p01"] = t
                        elif b == 3:
                            t = smx_p.tile([P, FW], bf16, name=f"{rp}p23_{mc}",
                                           tag="p23")
                            nc.gpsimd.tensor_add(t, fexp[2], fexp[3])
                            nc.gpsimd.tensor_add(tree["p01"], tree["p01"], t)
                        elif b == 5:
                            t = smx_p.tile([P, FW], bf16, name=f"{rp}p45_{mc}",
                                           tag="p45")
                            nc.gpsimd.tensor_add(t, fexp[4], fexp[5])
                            nc.gpsimd.tensor_add(tree["p01"], tree["p01"], t)
                        elif b == 7:
                            t = smx_p.tile([P, FW], bf16, name=f"{rp}p67_{mc}",
                                           tag="p67")
                            nc.vector.tensor_add(t, fexp[6], fexp[7])
                            S = smx_p.tile([P, FW], bf16, name=f"{rp}S{mc}",
                                           tag="S")
                            nc.vector.tensor_add(S, tree["p01"], t)
                            tree["S"] = S
                        if b >= DEN_LAG and den_q:
                            emit_den_b(*den_q.popleft())

                    def make_tail(fexp=fexp, tree=tree, mc=mc):
                        # R = 1/S as exp(-ln S) on Act (native DVE reciprocal
                        # measures ~13us/op on HW - useless), split into
                        # len(TAIL_SLOTS) column pieces; fh = fexp * R
                        # in-place on DVE, after the last piece by default
                        npiece = len(TAIL_SLOTS)
                        W = FW // npiece
                        lnS = smx_p.tile([P, FW], bf16, name=f"{rp}lnS{mc}",
                                         tag="lnS")
                        R = smx_p.tile([P, FW], bf16, name=f"{rp}R{mc}",
                                       tag="R")

                        def muls():
                            for b in range(B):
                                nc.vector.tensor_mul(fexp[b], fexp[b], R)

                        def tail(piece):
                            S = tree["S"]
                            cs = slice(piece * W, (piece + 1) * W)
                            nc.scalar.activation(lnS[:, cs], S[:, cs], AF.Ln)
                            nc.scalar.activation(R[:, cs], lnS[:, cs], AF.Exp,
                                                 scale=-1.0)
                            if piece == npiece - 1 and MUL_SLOT is None:
                                muls()
                        return tail, muls

                    prev_tail, prev_muls = make_tail()
                    for b in range(B):
                        den_q.append((mc, fexp, xt_t, b))

                for piece in range(len(TAIL_SLOTS)):
                    prev_tail(piece)
                if MUL_SLOT is not None:
                    prev_muls()
                while den_q:
                    emit_den_b(*den_q.popleft())

            for _rep in range(reps):
                emit_rep(f"r{_rep}_" if reps > 1 else "")

    return nc


def _split_excess_waits(nc, mybir, cap=1):
    """The installed walrus rejects engine instructions carrying more than
    one semaphore wait (setupSyncWait: "Too many sync wait commands"), but
    Tile's sem-assignment emits up to 4.  Legalize post-hoc: merge same-sem
    waits (max value), keep one on the instruction, and hoist the rest onto
    single-wait EventSemaphore instructions inserted just before, on the
    same engine queue (applies to every opcode incl. DMA pseudo-ops)."""
    n_ev = 0
    for fn in nc.m.functions:
        for blk in fn.blocks:
            insts = blk.instructions
            out = []
            changed = False
            for i in insts:
                si = getattr(i, "sync_info", None)
                waits = list(si.on_wait) if si is not None and si.on_wait else []
                if len(waits) > 1:
                    merged = {}
                    for w in waits:
                        k = w.id
                        if k not in merged or merged[k].wait_value < w.wait_value:
                            merged[k] = w
                    waits = list(merged.values())
                    while len(waits) > cap:
                        w = waits.pop(0)
                        ev = mybir.InstEventSemaphore(
                            name=f"{i.name}-wsplit{n_ev}", engine=i.engine)
                        ev.sync_info = mybir.SyncInfo(on_wait=[w], on_update=[])
                        try:
                            ev.debug = i.debug
                        except Exception:
                            pass
                        out.append(ev)
                        n_ev += 1
                    si.on_wait = waits
                    changed = True
                out.append(i)
            if changed:
                blk.instructions = out
    return n_ev


def _to_bf16(a):
    import ml_dtypes
    return np.ascontiguousarray(a).astype(ml_dtypes.bfloat16)


def _host_prep(x, w_theta, b_theta, w_phi, b_phi, w_conv, b_conv):
    x = np.asarray(x, dtype=np.float32)
    w_theta = np.asarray(w_theta, dtype=np.float32)
    b_theta = np.asarray(b_theta, dtype=np.float32)
    w_phi = np.asarray(w_phi, dtype=np.float32)
    b_phi = np.asarray(b_phi, dtype=np.float32)
    w_conv = np.asarray(w_conv, dtype=np.float32)
    b_conv = np.asarray(b_conv, dtype=np.float32)

    xr = x.reshape(B, C, HW)
    xb = _to_bf16(xr)                                  # [B, C, HW] bf16
    xtr = _to_bf16(xr.transpose(0, 2, 1))              # [B, HW, C] bf16
    xtr = np.ascontiguousarray(xtr).reshape(B, NCHUNK, 4, P, C)
    # 1/sqrt(C) = 1/16: exact power-of-two scale folded into theta
    wthT = _to_bf16((w_theta * (1.0 / 16.0)).T)
    wphT = _to_bf16(w_phi.T)
    wcT = np.ascontiguousarray((np.eye(C, dtype=np.float32) + w_conv).T)
    bth = np.ascontiguousarray((b_theta * (1.0 / 16.0)).reshape(D, 1))
    bph = np.ascontiguousarray(b_phi.reshape(D, 1))
    bc = np.ascontiguousarray(b_conv.reshape(C, 1))

    in_maps = []
    for k in range(NCORES):
        xs_k = np.ascontiguousarray(
            xb[:, :, k * NLOC:(k + 1) * NLOC]).reshape(B, 2, P, NLOC)
        in_maps.append({
            "xs": xs_k, "xn": xb.reshape(B, 2, P, HW), "xt": xtr,
            "wthT": wthT, "wphT": wphT, "wcT": wcT,
            "bth": bth, "bph": bph, "bc": bc,
        })
    return in_maps


def kernel(x, w_theta, b_theta, w_phi, b_phi, w_conv, b_conv):
    global _prog
    _ensure_path()
    from concourse.bass_utils import run_bass_kernel_spmd

    if _prog is None:
        _prog = _build()
        from concourse import mybir
        _split_excess_waits(_prog, mybir)

    in_maps = _host_prep(x, w_theta, b_theta, w_phi, b_phi, w_conv, b_conv)
    extra = {}
    if TRACE_CORES:
        extra["trace_cores"] = TRACE_CORES
    if TRACE_DIR:
        extra["tmpdir"] = TRACE_DIR
    res = run_bass_kernel_spmd(
        _prog, in_maps, list(range(NCORES)), trace=TRACE, **extra,
    )
    LAST["res"] = res

    outf = np.empty((B, C, HW), dtype=np.float32)
    for k in range(NCORES):
        outf[:, :, k * NLOC:(k + 1) * NLOC] = res.results[k]["out"].reshape(B, C, NLOC)
    return outf.reshape(B, C, 64, 64)


# revision 34
# speedup vs baseline: 1.2101x; 1.2101x over previous
"""Trainium2 Bass kernel for the DenoisingModule (non-local attention block).

Math (see reference):
    theta = Wt @ x + bt            [B, 128, HW]
    phi   = Wp @ x + bp            [B, 128, HW]
    f     = theta^T @ phi / 16     [B, HW, HW]
    fh    = softmax(f, axis=0)     (over the BATCH axis - PyTorch legacy dim=0)
    den   = fh @ x^T               [B, C, HW]
    out   = den + (Wc @ den + bc)  = (I + Wc) @ den + bc

Sharding: the softmax couples all 8 batch elements at each (n, m) position,
so batch-parallel would need a 64MB cross-device all-reduce.  Instead we
shard the *n* axis (rows of f / output pixels): each of the 8 cores owns
n in [k*512, (k+1)*512), holds full x, and the softmax is fully local.
No collectives at all; host slices inputs and concatenates outputs.

v3 (this file), on top of the bf16 datapath of v2:
  - softmax reduction tree moved to the otherwise-idle Pool engine
    (pairs 0-2 and their combines; only the late pair 6+7 and the final
    add stay on DVE), full-width [128,2048] ops;
  - the fh=fexp*R muls and the ln/exp reciprocal run full-width;
  - den matmuls for chunk k are emitted with a 2-slot lag into chunk
    k+1's stream so PE never stalls on the softmax tail;
  - startup DMAs are pair-batched (xs+xn interleaved per batch pair)
    and phi pair 0 is hoisted before theta pairs 1-3 so the first f
    matmul fires ~8us earlier;
  - conv bias-adds moved from DVE to Act (idle at the tail).
Engine budget per m-chunk: Act ~25us (16 exps + ln/exp + phi copies),
DVE ~22us (muls + den spills + late tree), PE ~24us (f + den + phi),
Pool ~21us (early tree).

The installed walrus rejects any engine/DMA instruction carrying more
than one semaphore wait ("Too many sync wait commands"), but Tile's
sem-assignment emits up to 4.  _split_excess_waits() legalizes the
scheduled program post-hoc by hoisting excess waits onto single-wait
EventSemaphore instructions inserted just before, on the same engine
queue (applied on the hardware path only; CoreSim runs the pre-split
program).
"""

import sys

import numpy as np

B = 8
C = 256
D = C // 2  # 128
HW = 4096
NCORES = 8
NLOC = HW // NCORES  # 512 n-columns per core
MC = 512  # m-chunk size
NCHUNK = HW // MC  # 8
P = 128

TRACE = False
TRACE_CORES = None
TRACE_DIR = None
LAST = {}

# pool-size knobs
XN_BUFS = 4    # paired [P,2,2,MC] tiles
XT_BUFS = 10
PHI_BUFS = 2
FEXP_BUFS = 2
PSA_BUFS = 2
PSD_BUFS = 2
OUT_BUFS = 2
DEN_LAG = 2    # den(c,b) emitted at slot b+DEN_LAG of chunk c+1
# scheduling knobs
TAIL_SLOTS = (0,)      # slots where LN/R pieces are emitted (1 or 2 pieces)
PHI_DVE = ()       # phi pairs whose copy runs on DVE (rest on Act)
MUL_SLOT = None        # slot where the muls are emitted (None: with last LN/R)
PHI_DEFER = ()         # phi pairs emitted after slot 0's exps (rest up front)

_prog = None


def _ensure_path():
    try:
        import concourse  # noqa: F401
    except ImportError:
        for p in ("/opt/trn_rl_repo", "/root/.axon_site/_ro/trn_rl_repo"):
            if p not in sys.path:
                sys.path.insert(0, p)
        import concourse  # noqa: F401


def _build(reps=1):
    from contextlib import ExitStack

    import concourse.bass as bass
    import concourse.tile as tile
    from concourse import mybir

    f32 = mybir.dt.float32
    f32r = mybir.dt.float32r
    bf16 = mybir.dt.bfloat16
    AF = mybir.ActivationFunctionType

    nc = bass.Bass(trn_type="TRN2", target_bir_lowering=False, debug=False)

    xs_h = nc.dram_tensor("xs", [B, 2, P, NLOC], bf16, kind="ExternalInput")
    xn_h = nc.dram_tensor("xn", [B, 2, P, HW], bf16, kind="ExternalInput")
    # xt laid out [B, chunk, s, p, c] so one DMA fetches a whole chunk
    xt_h = nc.dram_tensor("xt", [B, NCHUNK, 4, P, C], bf16, kind="ExternalInput")
    wthT_h = nc.dram_tensor("wthT", [C, D], bf16, kind="ExternalInput")
    wphT_h = nc.dram_tensor("wphT", [C, D], bf16, kind="ExternalInput")
    wcT_h = nc.dram_tensor("wcT", [C, C], f32r, kind="ExternalInput")
    bth_h = nc.dram_tensor("bth", [D, 1], f32, kind="ExternalInput")
    bph_h = nc.dram_tensor("bph", [D, 1], f32, kind="ExternalInput")
    bc_h = nc.dram_tensor("bc", [C, 1], f32, kind="ExternalInput")
    out_h = nc.dram_tensor("out", [B, 2, P, NLOC], f32, kind="ExternalOutput")

    FW = 4 * NLOC  # 2048: full free width of an fexp tile

    with tile.TileContext(nc) as tc:
        with ExitStack() as ctx:
            consts = ctx.enter_context(tc.tile_pool(name="consts", bufs=1))
            theta_p = ctx.enter_context(tc.tile_pool(name="theta", bufs=1))
            xs_p = ctx.enter_context(tc.tile_pool(name="xsp", bufs=2))
            xn_p = ctx.enter_context(tc.tile_pool(name="xnp", bufs=XN_BUFS))
            xt_p = ctx.enter_context(tc.tile_pool(name="xtp", bufs=XT_BUFS))
            phi_p = ctx.enter_context(tc.tile_pool(name="phip", bufs=PHI_BUFS))
            fexp_p = ctx.enter_context(tc.tile_pool(name="fexpp", bufs=FEXP_BUFS))
            smx_p = ctx.enter_context(tc.tile_pool(name="smxp", bufs=1))
            den_p = ctx.enter_context(tc.tile_pool(name="denp", bufs=1))
            out_p = ctx.enter_context(tc.tile_pool(name="outp", bufs=OUT_BUFS))
            psA = ctx.enter_context(tc.tile_pool(name="psA", bufs=PSA_BUFS, space="PSUM"))
            psD = ctx.enter_context(tc.tile_pool(name="psD", bufs=PSD_BUFS, space="PSUM"))

            # ---- constants (emitted in dependency-criticality order; the
            # first xs/xn pair DMAs are interleaved by emit_rep before the
            # conv constants) ----
            wth_sb = []
            wph_sb = []
            wc_sb = []

            def emit_theta_consts():
                # biases first: they are tiny and gate the first Act copies
                t = consts.tile([D, 1], f32, name="bth", tag="bth")
                nc.sync.dma_start(out=t, in_=bth_h.ap()[:, :])
                t2 = consts.tile([D, 1], f32, name="bph", tag="bph")
                nc.sync.dma_start(out=t2, in_=bph_h.ap()[:, :])
                cst["bph"] = t2
                for ck in range(2):
                    w = consts.tile([P, D], bf16, name=f"wth{ck}", tag=f"wth{ck}")
                    nc.sync.dma_start(out=w, in_=wthT_h.ap()[ck * P:(ck + 1) * P, :])
                    wth_sb.append(w)
                return t

            def emit_phi_consts():
                for ck in range(2):
                    t = consts.tile([P, D], bf16, name=f"wph{ck}", tag=f"wph{ck}")
                    nc.sync.dma_start(out=t, in_=wphT_h.ap()[ck * P:(ck + 1) * P, :])
                    wph_sb.append(t)
                return cst["bph"]

            def emit_conv_consts():
                bc_sb = []
                for ck in range(2):
                    t = consts.tile([P, C], f32r, name=f"wc{ck}", tag=f"wc{ck}")
                    nc.sync.dma_start(out=t, in_=wcT_h.ap()[ck * P:(ck + 1) * P, :])
                    wc_sb.append(t)
                for dk in range(2):
                    t = consts.tile([P, 1], f32, name=f"bc{dk}", tag=f"bc{dk}")
                    nc.sync.dma_start(out=t, in_=bc_h.ap()[dk * P:(dk + 1) * P, :])
                    bc_sb.append(t)
                return bc_sb

            cst = {}

            def emit_rep(rp):
                theta_sb = [None] * B
                xn0_tiles = [None] * 4  # per pair

                def emit_theta_pair(p, first=False):
                    xst = xs_p.tile([P, 2, 2, NLOC], bf16, name=f"{rp}xs{p}", tag="xs")
                    nc.sync.dma_start(
                        out=xst,
                        in_=xs_h.ap()[2 * p:2 * p + 2].transpose([2, 0, 1, 3]))
                    xnt = xn_p.tile([P, 2, 2, MC], bf16, name=f"{rp}xn0_{p}", tag="xn")
                    nc.sync.dma_start(
                        out=xnt,
                        in_=xn_h.ap()[2 * p:2 * p + 2, :, :, 0:MC]
                        .transpose([2, 0, 1, 3]))
                    xn0_tiles[p] = xnt
                    if first and not wth_sb:
                        cst["bth"] = emit_theta_consts()
                        cst["bph"] = emit_phi_consts()
                    ps = psA.tile([P, 2 * NLOC], f32, name=f"{rp}psth{p}", tag="psA")
                    for ck in range(2):
                        for bi in range(2):
                            nc.tensor.matmul(
                                ps[:, bi * NLOC:(bi + 1) * NLOC],
                                wth_sb[ck], xst[:, bi, ck, :],
                                start=(ck == 0), stop=(ck == 1))
                    for bi in range(2):
                        b = 2 * p + bi
                        th = theta_p.tile([D, NLOC], bf16, name=f"{rp}theta{b}",
                                          tag=f"theta{b}")
                        nc.scalar.activation(th, ps[:, bi * NLOC:(bi + 1) * NLOC],
                                             AF.Identity, bias=cst["bth"])
                        theta_sb[b] = th

                den_sb = [None] * B

                def emit_conv(b):
                    # out = (I + Wc) @ den + bc  (f32r matmul, bias-add on Act)
                    ot = out_p.tile([P, 2, NLOC], f32, name=f"{rp}out{b}", tag="out")
                    for dk in range(2):
                        ps = psA.tile([P, 2 * NLOC], f32, name=f"{rp}pso{b}_{dk}",
                                      tag="psA")
                        for ct in range(2):
                            nc.tensor.matmul(
                                ps[:, :NLOC],
                                wc_sb[ct][:, dk * P:(dk + 1) * P],
                                den_sb[b][:, ct * NLOC:(ct + 1) * NLOC],
                                start=(ct == 0), stop=(ct == 1))
                        nc.scalar.activation(ot[:, dk, :], ps[:, :NLOC],
                                             AF.Identity, bias=cst["bc"][dk])
                    nc.sync.dma_start(out=out_h.ap()[b].transpose([1, 0, 2]), in_=ot)

                def emit_den_b(mc, fexp, xt_t, b):
                    psd = psD.tile([P, 2 * NLOC], f32, name=f"{rp}psd{mc}_{b}",
                                   tag="psD")
                    for ct in range(2):
                        for s in range(4):
                            nc.tensor.matmul(
                                psd[:, ct * NLOC:(ct + 1) * NLOC],
                                xt_t[b][:, s, ct * P:(ct + 1) * P],
                                fexp[b][:, s * NLOC:(s + 1) * NLOC],
                                start=(s == 0), stop=(s == 3))
                    if mc == 0:
                        dn = den_p.tile([P, 2 * NLOC], f32r, name=f"{rp}den{b}",
                                        tag=f"den{b}")
                        nc.vector.tensor_copy(dn, psd)
                        den_sb[b] = dn
                    else:
                        nc.vector.tensor_add(den_sb[b], den_sb[b], psd)
                    if mc == NCHUNK - 1:
                        emit_conv(b)

                # startup: pair 0's big DMAs go first, the small constant
                # DMAs are interleaved right behind them, conv constants
                # after theta pair 0 (needed only at the tail).
                emit_theta_pair(0, first=True)
                if "bc" not in cst:
                    cst["bc"] = emit_conv_consts()

                from collections import deque
                den_q = deque()   # (mc, fexp, xt_t, b) awaiting den emission
                prev_tail = None  # emits LN/R piece of the previous chunk
                prev_muls = None  # emits fexp*R muls of the previous chunk

                for mc in range(NCHUNK):
                    m0 = mc * MC
                    phi_sb = []

                    def emit_phi_pair(bp, mc=mc, m0=m0):
                        ps = psA.tile([P, 2 * NLOC], f32, name=f"{rp}psph{mc}_{bp}",
                                      tag="psA")
                        if mc == 0:
                            xnt = xn0_tiles[bp]
                        else:
                            xnt = xn_p.tile([P, 2, 2, MC], bf16,
                                            name=f"{rp}xn{mc}_{bp}", tag="xn")
                            nc.sync.dma_start(
                                out=xnt,
                                in_=xn_h.ap()[2 * bp:2 * bp + 2, :, :, m0:m0 + MC]
                                .transpose([2, 0, 1, 3]))
                        for ck in range(2):
                            for bi in range(2):
                                nc.tensor.matmul(
                                    ps[:, bi * MC:(bi + 1) * MC], wph_sb[ck],
                                    xnt[:, bi, ck, :],
                                    start=(ck == 0), stop=(ck == 1))
                        php = phi_p.tile([D, 2, MC], bf16, name=f"{rp}phi{mc}_{bp}",
                                         tag=f"phi{bp}")
                        # copies split across DVE / Act per the PHI_DVE knob
                        if bp in PHI_DVE:
                            nc.vector.tensor_scalar(php, ps, cst["bph"], None,
                                                    mybir.AluOpType.add)
                        else:
                            nc.scalar.activation(php, ps, AF.Identity,
                                                 bias=cst["bph"])
                        phi_sb.append(php)

                    if mc > 0:
                        # phi pairs up front except PHI_DEFER, whose Act
                        # copies interleave with the exp stream instead of
                        # fronting it
                        for bp in range(4):
                            if bp not in PHI_DEFER:
                                emit_phi_pair(bp)
                        # two oldest queued dens (chunk mc-2's trailing pair):
                        # PE chews these while the phi copies drain.  Only
                        # when the queue holds >8 (i.e. the head is from two
                        # chunks back, whose muls are long emitted)
                        while len(den_q) > 8:
                            emit_den_b(*den_q.popleft())

                    # f~ = theta'^T phi, exp -> fexp [m=128, (s,n) free], bf16.
                    # Softmax tree rides the exp stream: Pool sums pairs 0-2
                    # and combines them; DVE only does the late pair 6+7 and
                    # the final add.  The previous chunk's LN/R + muls are
                    # emitted after slot 0's exps so the next exp stream isn't
                    # queued behind them; den matmuls of the previous chunk
                    # lag DEN_LAG slots so their fexp*R inputs are ready.
                    fexp = []
                    tree = {}
                    xt_t = []
                    for b in range(B):
                        if mc == 0 and b % 2 == 0:
                            emit_phi_pair(b // 2)
                            if b < 6:
                                emit_theta_pair(b // 2 + 1)
                        fe = fexp_p.tile([P, FW], bf16, name=f"{rp}fexp{mc}_{b}",
                                         tag=f"fexp{b}")
                        for sp in range(2):
                            ps = psA.tile([P, 2 * NLOC], f32,
                                          name=f"{rp}psf{mc}_{b}_{sp}", tag="psA")
                            for si in range(2):
                                s = sp * 2 + si
                                nc.tensor.matmul(
                                    ps[:, si * NLOC:(si + 1) * NLOC],
                                    phi_sb[b // 2][:, b % 2, s * P:(s + 1) * P],
                                    theta_sb[b],
                                    start=True, stop=True)
                            nc.scalar.activation(
                                fe[:, sp * 2 * NLOC:(sp + 1) * 2 * NLOC], ps,
                                AF.Exp)
                        fexp.append(fe)
                        if mc > 0 and b == 0:
                            for bp in PHI_DEFER:
                                emit_phi_pair(bp)
                        # xT tile for this (chunk, batch), spread across slots
                        t = xt_p.tile([P, 4, C], bf16, name=f"{rp}xt{mc}_{b}",
                                      tag="xt")
                        nc.sync.dma_start(
                            out=t, in_=xt_h.ap()[b, mc].transpose([1, 0, 2]))
                        xt_t.append(t)
                        for piece, slot in enumerate(TAIL_SLOTS):
                            if b == slot and prev_tail is not None:
                                prev_tail(piece)
                        if MUL_SLOT is not None and b == MUL_SLOT \
                                and prev_muls is not None:
                            prev_muls()
                        # softmax reduction tree, full-width on DVE.  Pool
                        # (GpSimd) is NOT used: concurrent Pool ops contend
                        # with DVE on SBUF and blow DVE op latency up 2-4x
                        # (measured: muls 1211ns clean -> 3203ns avg with the
                        # tree on Pool).
                        if b == 1:
                            t = smx_p.tile([P, FW], bf16, name=f"{rp}p01_{mc}",
                                           tag="p01")
                            nc.vector.tensor_add(t, fexp[0], fexp[1])
                            tree["p01"] = t
                        elif b == 3:
                            t = smx_p.tile([P, FW], bf16, name=f"{rp}p23_{mc}",
                                           tag="p23")
                            nc.vector.tensor_add(t, fexp[2], fexp[3])
                            nc.vector.tensor_add(tree["p01"], tree["p01"], t)
                        elif b == 5:
                            t = smx_p.tile([P, FW], bf16, name=f"{rp}p45_{mc}",
                                           tag="p45")
                            nc.vector.tensor_add(t, fexp[4], fexp[5])
                            nc.vector.tensor_add(tree["p01"], tree["p01"], t)
                        elif b == 7:
                            t = smx_p.tile([P, FW], bf16, name=f"{rp}p67_{mc}",
                                           tag="p67")
                            nc.vector.tensor_add(t, fexp[6], fexp[7])
                            S = smx_p.tile([P, FW], bf16, name=f"{rp}S{mc}",
                                           tag="S")
                            nc.vector.tensor_add(S, tree["p01"], t)
                            tree["S"] = S
                        if b >= DEN_LAG and den_q:
                            emit_den_b(*den_q.popleft())

                    def make_tail(fexp=fexp, tree=tree, mc=mc):
                        # R = 1/S as exp(-ln S) on Act (native DVE reciprocal
                        # measures ~13us/op on HW - useless), split into
                        # len(TAIL_SLOTS) column pieces; fh = fexp * R
                        # in-place on DVE, after the last piece by default
                        npiece = len(TAIL_SLOTS)
                        W = FW // npiece
                        lnS = smx_p.tile([P, FW], bf16, name=f"{rp}lnS{mc}",
                                         tag="lnS")
                        R = smx_p.tile([P, FW], bf16, name=f"{rp}R{mc}",
                                       tag="R")

                        def muls():
                            for b in range(B):
                                nc.vector.tensor_mul(fexp[b], fexp[b], R)

                        def tail(piece):
                            S = tree["S"]
                            cs = slice(piece * W, (piece + 1) * W)
                            nc.scalar.activation(lnS[:, cs], S[:, cs], AF.Ln)
                            nc.scalar.activation(R[:, cs], lnS[:, cs], AF.Exp,
                                                 scale=-1.0)
                            if piece == npiece - 1 and MUL_SLOT is None:
                                muls()
                        return tail, muls

                    prev_tail, prev_muls = make_tail()
                    for b in range(B):
                        den_q.append((mc, fexp, xt_t, b))

                for piece in range(len(TAIL_SLOTS)):
                    prev_tail(piece)
                if MUL_SLOT is not None:
                    prev_muls()
                while den_q:
                    emit_den_b(*den_q.popleft())

            for _rep in range(reps):
                emit_rep(f"r{_rep}_" if reps > 1 else "")

    return nc


def _split_excess_waits(nc, mybir, cap=1):
    """The installed walrus rejects engine instructions carrying more than
    one semaphore wait (setupSyncWait: "Too many sync wait commands"), but
    Tile's sem-assignment emits up to 4.  Legalize post-hoc: merge same-sem
    waits (max value), keep one on the instruction, and hoist the rest onto
    single-wait EventSemaphore instructions inserted just before, on the
    same engine queue (applies to every opcode incl. DMA pseudo-ops)."""
    n_ev = 0
    for fn in nc.m.functions:
        for blk in fn.blocks:
            insts = blk.instructions
            out = []
            changed = False
            for i in insts:
                si = getattr(i, "sync_info", None)
                waits = list(si.on_wait) if si is not None and si.on_wait else []
                if len(waits) > 1:
                    merged = {}
                    for w in waits:
                        k = w.id
                        if k not in merged or merged[k].wait_value < w.wait_value:
                            merged[k] = w
                    waits = list(merged.values())
                    while len(waits) > cap:
                        w = waits.pop(0)
                        ev = mybir.InstEventSemaphore(
                            name=f"{i.name}-wsplit{n_ev}", engine=i.engine)
                        ev.sync_info = mybir.SyncInfo(on_wait=[w], on_update=[])
                        try:
                            ev.debug = i.debug
                        except Exception:
                            pass
                        out.append(ev)
                        n_ev += 1
                    si.on_wait = waits
                    changed = True
                out.append(i)
            if changed:
                blk.instructions = out
    return n_ev


def _to_bf16(a):
    import ml_dtypes
    return np.ascontiguousarray(a).astype(ml_dtypes.bfloat16)


def _host_prep(x, w_theta, b_theta, w_phi, b_phi, w_conv, b_conv):
    x = np.asarray(x, dtype=np.float32)
    w_theta = np.asarray(w_theta, dtype=np.float32)
    b_theta = np.asarray(b_theta, dtype=np.float32)
    w_phi = np.asarray(w_phi, dtype=np.float32)
    b_phi = np.asarray(b_phi, dtype=np.float32)
    w_conv = np.asarray(w_conv, dtype=np.float32)
    b_conv = np.asarray(b_conv, dtype=np.float32)

    xr = x.reshape(B, C, HW)
    xb = _to_bf16(xr)                                  # [B, C, HW] bf16
    xtr = _to_bf16(xr.transpose(0, 2, 1))              # [B, HW, C] bf16
    xtr = np.ascontiguousarray(xtr).reshape(B, NCHUNK, 4, P, C)
    # 1/sqrt(C) = 1/16: exact power-of-two scale folded into theta
    wthT = _to_bf16((w_theta * (1.0 / 16.0)).T)
    wphT = _to_bf16(w_phi.T)
    wcT = np.ascontiguousarray((np.eye(C, dtype=np.float32) + w_conv).T)
    bth = np.ascontiguousarray((b_theta * (1.0 / 16.0)).reshape(D, 1))
    bph = np.ascontiguousarray(b_phi.reshape(D, 1))
    bc = np.ascontiguousarray(b_conv.reshape(C, 1))

    in_maps = []
    for k in range(NCORES):
        xs_k = np.ascontiguousarray(
            xb[:, :, k * NLOC:(k + 1) * NLOC]).reshape(B, 2, P, NLOC)
        in_maps.append({
            "xs": xs_k, "xn": xb.reshape(B, 2, P, HW), "xt": xtr,
            "wthT": wthT, "wphT": wphT, "wcT": wcT,
            "bth": bth, "bph": bph, "bc": bc,
        })
    return in_maps


def kernel(x, w_theta, b_theta, w_phi, b_phi, w_conv, b_conv):
    global _prog
    _ensure_path()
    from concourse.bass_utils import run_bass_kernel_spmd

    if _prog is None:
        _prog = _build()
        from concourse import mybir
        _split_excess_waits(_prog, mybir)

    in_maps = _host_prep(x, w_theta, b_theta, w_phi, b_phi, w_conv, b_conv)
    extra = {}
    if TRACE_CORES:
        extra["trace_cores"] = TRACE_CORES
    if TRACE_DIR:
        extra["tmpdir"] = TRACE_DIR
    res = run_bass_kernel_spmd(
        _prog, in_maps, list(range(NCORES)), trace=TRACE, **extra,
    )
    LAST["res"] = res

    outf = np.empty((B, C, HW), dtype=np.float32)
    for k in range(NCORES):
        outf[:, :, k * NLOC:(k + 1) * NLOC] = res.results[k]["out"].reshape(B, C, NLOC)
    return outf.reshape(B, C, 64, 64)
